# revision 3
# baseline (speedup 1.0000x reference)
"""Causal self-attention block (B=4, T=2048, C=1024, H=16) on 8 TRN2 NeuronCores.

Sharding: core -> (batch b = core//2, head-group g = core%2, 8 heads each).
Each core computes, for its (b, g):
    qkv slice -> per-head causal softmax(Q K^T * scale) V -> partial out^T =
    W_proj[g-rows]^T @ Y^T  (f32 [1024, 2048])
Host sums the two group partials per batch, transposes, adds b_proj.

All matmuls in bf16 (fp32 accumulation in PSUM).
"""

import numpy as np
import ml_dtypes

import concourse.bass as bass
import concourse.mybir as mybir
import concourse.tile as tile
from concourse import bacc
from concourse.bass_utils import run_bass_kernel_spmd

BF16 = mybir.dt.bfloat16
F32 = mybir.dt.float32

B, T, C = 4, 2048, 1024
H = 16          # total heads
HD = 64         # head dim
HPC = 8         # heads per core
G = 2           # head groups (cores per batch)
FQK = 512       # q (or k) features per core
FV = 512        # v features per core
VE = 65         # v features + ones column, per head
SCALE = float(0.1 / np.sqrt(np.float32(HD)))

N_CT = 8        # c chunks of 128
N_TT = 16       # t tiles of 128
N_TB = 4        # t blocks of 512
N_HP = 4        # head pairs per core
N_IB = 4        # i blocks of 512


def build_nc(loop_n: int = 1) -> bacc.Bacc:
    nc = bacc.Bacc("TRN2", target_bir_lowering=False, debug=False, num_devices=8)

    xt = nc.dram_tensor("xt", [C, T], BF16, kind="ExternalInput")
    wt = nc.dram_tensor("wt", [C, 3 * FQK], BF16, kind="ExternalInput")
    wb = nc.dram_tensor("wb", [1, 3 * FQK], BF16, kind="ExternalInput")
    wp = nc.dram_tensor("wp", [FV, C], BF16, kind="ExternalInput")
    outT = nc.dram_tensor("outT", [C, T], F32, kind="ExternalOutput")

    with tile.TileContext(nc) as tc:
        from contextlib import ExitStack

        with ExitStack() as ctx:
            if loop_n > 1:
                ctx.enter_context(tc.For_i(0, loop_n, 1))
            consts = ctx.enter_context(tc.tile_pool(name="consts", bufs=1))
            p_xt = ctx.enter_context(tc.tile_pool(name="p_xt", bufs=1))
            p_wt = ctx.enter_context(tc.tile_pool(name="p_wt", bufs=1))
            p_wp = ctx.enter_context(tc.tile_pool(name="p_wp", bufs=1))
            p_qkt = ctx.enter_context(tc.tile_pool(name="p_qkt", bufs=1))
            p_vex = ctx.enter_context(tc.tile_pool(name="p_vex", bufs=1))
            p_yt = ctx.enter_context(tc.tile_pool(name="p_yt", bufs=1))
            p_ex = ctx.enter_context(tc.tile_pool(name="p_ex", bufs=3))
            p_small = ctx.enter_context(tc.tile_pool(name="p_small", bufs=4))
            p_stage = ctx.enter_context(tc.tile_pool(name="p_stage", bufs=2))
            ps_qkv = ctx.enter_context(
                tc.tile_pool(name="ps_qkv", bufs=2, space="PSUM")
            )
            ps_s = ctx.enter_context(tc.tile_pool(name="ps_s", bufs=2, space="PSUM"))
            ps_pv = ctx.enter_context(tc.tile_pool(name="ps_pv", bufs=1, space="PSUM"))

            # ---- constants ----
            ones = consts.tile([1, 512], BF16, tag="ones")
            nc.gpsimd.memset(ones[:], 1.0)
            # causal keep-mask for a diagonal [j=128, i=128] tile:
            # mask[p, f] = 1.0 if f - p >= 0 else 0.0
            maskc = consts.tile([128, 128], BF16, tag="maskc")
            nc.gpsimd.memset(maskc[:], 1.0)
            nc.gpsimd.affine_select(
                out=maskc[:],
                in_=maskc[:],
                compare_op=mybir.AluOpType.is_ge,
                fill=0.0,
                base=0,
                channel_multiplier=-1,
                pattern=[[1, 128]],
            )

            # ---- input DMAs ----
            XT = []
            for c in range(N_CT):
                t_ = p_xt.tile([128, T], BF16, tag=f"xt{c}")
                nc.sync.dma_start(t_[:], xt[c * 128 : (c + 1) * 128, :])
                XT.append(t_)
            WT = []
            for c in range(N_CT):
                t_ = p_wt.tile([128, 3 * FQK], BF16, tag=f"wt{c}")
                nc.sync.dma_start(t_[:], wt[c * 128 : (c + 1) * 128, :])
                WT.append(t_)
            WB = p_wt.tile([1, 3 * FQK], BF16, tag="wb")
            nc.sync.dma_start(WB[:], wb[:, :])
            WP = []
            for d in range(4):
                t_ = p_wp.tile([128, C], BF16, tag=f"wp{d}")
                nc.sync.dma_start(t_[:], wp[d * 128 : (d + 1) * 128, :])
                WP.append(t_)

            # ---- V natural [t, 512] -> V_ext [t, 8*65] (ones col per head) ----
            VEX = []
            for tt in range(N_TT):
                vex = p_vex.tile([128, HPC * VE], BF16, tag=f"vex{tt}")
                nc.gpsimd.memset(vex[:], 1.0)
                ps = ps_qkv.tile([128, 512], F32, tag="qkv")
                for c in range(N_CT):
                    nc.tensor.matmul(
                        ps[:],
                        lhsT=XT[c][:, tt * 128 : (tt + 1) * 128],
                        rhs=WT[c][:, 2 * FQK : 3 * FQK],
                        start=(c == 0),
                        stop=False,
                    )
                nc.tensor.matmul(
                    ps[:],
                    lhsT=ones[0:1, 0:128],
                    rhs=WB[0:1, 2 * FQK : 3 * FQK],
                    start=False,
                    stop=True,
                )
                dst = vex[:].rearrange("p (h e) -> p h e", h=HPC)[:, :, 0:HD]
                src = ps[:].rearrange("p (h e) -> p h e", h=HPC)
                nc.vector.tensor_copy(dst, src)
                VEX.append(vex)

            # ---- per head pair: QKV^T tiles then attention ----
            QKT = [None] * 8  # f-tiles: 0-3 = Q features, 4-7 = K features
            YT = []
            for hp in range(N_HP):
                yt = p_yt.tile([128, T], BF16, tag=f"yt{hp}")
                YT.append(yt)

            for hp in range(N_HP):
                # QKV^T feature-major: out[f, t] = sum_c W[c, f] * XT[c, t]
                for ft in (hp, 4 + hp):
                    qkt = p_qkt.tile([128, T], BF16, tag=f"qkt{ft}")
                    QKT[ft] = qkt
                    for tb in range(N_TB):
                        ps = ps_qkv.tile([128, 512], F32, tag="qkv")
                        for c in range(N_CT):
                            nc.tensor.matmul(
                                ps[:],
                                lhsT=WT[c][:, ft * 128 : (ft + 1) * 128],
                                rhs=XT[c][:, tb * 512 : (tb + 1) * 512],
                                start=(c == 0),
                                stop=False,
                            )
                        nc.tensor.matmul(
                            ps[:],
                            lhsT=WB[0:1, ft * 128 : (ft + 1) * 128],
                            rhs=ones[0:1, 0:512],
                            start=False,
                            stop=True,
                        )
                        nc.vector.tensor_copy(
                            qkt[:, tb * 512 : (tb + 1) * 512], ps[:]
                        )

                qt = QKT[hp]
                kt = QKT[4 + hp]
                for ib in range(N_IB):
                    i_base = 512 * ib
                    njt = 4 * (ib + 1)
                    pvA = ps_pv.tile([VE, 512], F32, tag="pvA")
                    pvB = ps_pv.tile([VE, 512], F32, tag="pvB")
                    for jt in range(njt):
                        j_base = 128 * jt
                        i_lo = max(i_base, j_base)
                        w = i_base + 512 - i_lo
                        rel = i_lo - i_base
                        ps = ps_s.tile([128, 1024], F32, tag="s")
                        # S^T[j, i] for heads A (rows 0:64 of qkt/kt) and B
                        nc.tensor.matmul(
                            ps[:, rel : rel + w],
                            lhsT=kt[0:64, j_base : j_base + 128],
                            rhs=qt[0:64, i_lo : i_lo + w],
                            start=True,
                            stop=True,
                        )
                        nc.tensor.matmul(
                            ps[:, 512 + rel : 512 + rel + w],
                            lhsT=kt[64:128, j_base : j_base + 128],
                            rhs=qt[64:128, i_lo : i_lo + w],
                            start=True,
                            stop=True,
                        )
                        ex = p_ex.tile([128, 1024], BF16, tag="ex")
                        ps3 = ps[:].rearrange("p (b q) -> p b q", b=2)[
                            :, :, rel : rel + w
                        ]
                        ex3 = ex[:].rearrange("p (b q) -> p b q", b=2)[
                            :, :, rel : rel + w
                        ]
                        nc.scalar.activation(
                            ex3, ps3, mybir.ActivationFunctionType.Exp, scale=SCALE
                        )
                        if j_base >= i_base:
                            # diagonal tile: zero out j > i in [i_lo, i_lo+128)
                            for h2 in range(2):
                                sl = ex[:, 512 * h2 + rel : 512 * h2 + rel + 128]
                                nc.vector.tensor_mul(sl, sl, maskc[:])
                        for h2, pv in ((0, pvA), (1, pvB)):
                            nc.tensor.matmul(
                                pv[:, rel : rel + w],
                                lhsT=VEX[jt][
                                    :, (2 * hp + h2) * VE : (2 * hp + h2 + 1) * VE
                                ],
                                rhs=ex[:, 512 * h2 + rel : 512 * h2 + rel + w],
                                start=(jt == 0),
                                stop=(jt == njt - 1),
                                skip_group_check=True,
                            )
                    # normalize: rows 0:64 divided by row 64 (the ones-column sum)
                    for h2, pv in ((0, pvA), (1, pvB)):
                        rsf = p_small.tile([1, 512], F32, tag="rsf")
                        nc.vector.reciprocal(rsf[:], pv[HD : HD + 1, :])
                        rsb = p_small.tile([1, 512], BF16, tag="rsb")
                        nc.vector.tensor_copy(rsb[:], rsf[:])
                        bc = ps_qkv.tile([64, 512], F32, tag="qkv")
                        nc.tensor.matmul(
                            bc[:],
                            lhsT=ones[0:1, 0:64],
                            rhs=rsb[:],
                            start=True,
                            stop=True,
                        )
                        bcs = p_small.tile([64, 512], BF16, tag="bcs")
                        nc.vector.tensor_copy(bcs[:], bc[:])
                        nc.vector.tensor_mul(
                            YT[hp][
                                64 * h2 : 64 * h2 + 64,
                                i_base : i_base + 512,
                            ],
                            pv[0:HD, :],
                            bcs[:],
                        )

            # ---- out^T = wp^T @ YT  (co-major) ----
            for ct in range(8):
                stage = p_stage.tile([128, T], F32, tag="st")
                for tb in range(N_TB):
                    ps = ps_qkv.tile([128, 512], F32, tag="qkv")
                    for d in range(4):
                        nc.tensor.matmul(
                            ps[:],
                            lhsT=WP[d][:, ct * 128 : (ct + 1) * 128],
                            rhs=YT[d][:, tb * 512 : (tb + 1) * 512],
                            start=(d == 0),
                            stop=(d == 3),
                        )
                    nc.vector.tensor_copy(stage[:, tb * 512 : (tb + 1) * 512], ps[:])
                nc.sync.dma_start(outT[ct * 128 : (ct + 1) * 128, :], stage[:])

    nc.compile()
    return nc


_NC = None


def _get_nc():
    global _NC
    if _NC is None:
        _NC = build_nc()
    return _NC


def make_in_maps(x, W_attn, b_attn, W_proj):
    bf = ml_dtypes.bfloat16
    xt_b = [np.ascontiguousarray(x[b].astype(bf).T) for b in range(B)]
    in_maps = []
    for core in range(8):
        b, g = core // 2, core % 2
        wq = W_attn[:, 512 * g : 512 * g + 512]
        wk = W_attn[:, C + 512 * g : C + 512 * g + 512]
        wv = W_attn[:, 2 * C + 512 * g : 2 * C + 512 * g + 512]
        wt = np.ascontiguousarray(np.concatenate([wq, wk, wv], axis=1)).astype(bf)
        bq = b_attn[512 * g : 512 * g + 512]
        bk = b_attn[C + 512 * g : C + 512 * g + 512]
        bv = b_attn[2 * C + 512 * g : 2 * C + 512 * g + 512]
        wb_ = np.concatenate([bq, bk, bv])[None, :].astype(bf)
        wp_ = np.ascontiguousarray(W_proj[512 * g : 512 * g + 512, :]).astype(bf)
        in_maps.append({"xt": xt_b[b], "wt": wt, "wb": wb_, "wp": wp_})
    return in_maps


def combine_outputs(results, b_proj):
    out = np.empty((B, T, C), dtype=np.float32)
    for b in range(B):
        acc = results[2 * b]["outT"] + results[2 * b + 1]["outT"]
        out[b] = acc.T + b_proj[None, :]
    return out


def kernel(x, W_attn, b_attn, W_proj, b_proj):
    x = np.asarray(x, dtype=np.float32)
    W_attn = np.asarray(W_attn, dtype=np.float32)
    b_attn = np.asarray(b_attn, dtype=np.float32)
    W_proj = np.asarray(W_proj, dtype=np.float32)
    b_proj = np.asarray(b_proj, dtype=np.float32)

    nc = _get_nc()
    in_maps = make_in_maps(x, W_attn, b_attn, W_proj)
    res = run_bass_kernel_spmd(nc, in_maps, core_ids=list(range(8)))
    return combine_outputs(res.results, b_proj)


# revision 26
# speedup vs baseline: 2.0746x; 2.0746x over previous
"""Causal self-attention block (B=4, T=2048, C=1024, H=16) on 8 TRN2 NeuronCores.

Sharding: core -> (batch b = core//2, head-group g = core%2, 8 heads each).
Each core computes, for its (b, g):
    qkv slice -> per-head causal softmax(Q K^T * scale) V -> partial out^T =
    W_proj[g-rows]^T @ Y^T  (f32 [1024, 2048])
Host sums the two group partials per batch, transposes, adds b_proj.

All matmuls in bf16 (fp32 accumulation in PSUM).
"""

import numpy as np
import ml_dtypes

import concourse.bass as bass
import concourse.mybir as mybir
import concourse.tile as tile
from concourse import bacc
from concourse.bass_utils import run_bass_kernel_spmd

BF16 = mybir.dt.bfloat16
F32 = mybir.dt.float32

B, T, C = 4, 2048, 1024
H = 16          # total heads
HD = 64         # head dim
HPC = 8         # heads per core
G = 2           # head groups (cores per batch)
FQK = 512       # q (or k) features per core
FV = 512        # v features per core
VE = 65         # v features + ones column, per head
SCALE = float(0.1 / np.sqrt(np.float32(HD)))

N_CT = 8        # c chunks of 128
N_TT = 16       # t tiles of 128
N_TB = 4        # t blocks of 512
N_HP = 4        # head pairs per core
N_IB = 4        # i blocks of 512


def build_nc(loop_n: int = 1, ablate: str = "") -> bacc.Bacc:
    """ablate: comma tokens from {exp, s, pv, attn, mask} to skip stages (timing only)."""
    ab = set(t for t in ablate.split(",") if t)
    use_s = "s" not in ab and "attn" not in ab
    use_exp = "exp" not in ab and "attn" not in ab
    use_mask = "mask" not in ab and "attn" not in ab
    use_pv = "pv" not in ab and "attn" not in ab
    use_qkv = "qkv" not in ab
    use_proj = "proj" not in ab
    if not use_qkv:
        assert "attn" in ab, "qkv ablation requires attn ablation"
    nc = bacc.Bacc("TRN2", target_bir_lowering=False, debug=False, num_devices=8)

    xt = nc.dram_tensor("xt", [C, T], BF16, kind="ExternalInput")
    wt = nc.dram_tensor("wt", [C, 3 * FQK], BF16, kind="ExternalInput")
    wb = nc.dram_tensor("wb", [1, 3 * FQK], BF16, kind="ExternalInput")
    wp = nc.dram_tensor("wp", [FV, C], BF16, kind="ExternalInput")
    outT = nc.dram_tensor("outT", [C, T], F32, kind="ExternalOutput")

    with tile.TileContext(nc) as tc:
        from contextlib import ExitStack

        with ExitStack() as ctx:
            if loop_n > 1:
                ctx.enter_context(tc.For_i(0, loop_n, 1))
            consts = ctx.enter_context(tc.tile_pool(name="consts", bufs=1))
            p_xt = ctx.enter_context(tc.tile_pool(name="p_xt", bufs=1))
            p_wt = ctx.enter_context(tc.tile_pool(name="p_wt", bufs=1))
            p_wp = ctx.enter_context(tc.tile_pool(name="p_wp", bufs=1))
            p_qkt = ctx.enter_context(tc.tile_pool(name="p_qkt", bufs=1))
            p_vex = ctx.enter_context(tc.tile_pool(name="p_vex", bufs=1))
            p_yt = ctx.enter_context(tc.tile_pool(name="p_yt", bufs=1))
            p_ex = ctx.enter_context(tc.tile_pool(name="p_ex", bufs=4))
            p_small = ctx.enter_context(tc.tile_pool(name="p_small", bufs=4))
            p_stage = ctx.enter_context(tc.tile_pool(name="p_stage", bufs=2))
            ps_qkv = ctx.enter_context(
                tc.tile_pool(name="ps_qkv", bufs=2, space="PSUM")
            )
            ps_s = ctx.enter_context(tc.tile_pool(name="ps_s", bufs=2, space="PSUM"))
            ps_pv = ctx.enter_context(tc.tile_pool(name="ps_pv", bufs=1, space="PSUM"))

            # ---- constants ----
            ones = consts.tile([1, 512], BF16, tag="ones")
            nc.gpsimd.memset(ones[:], 1.0)
            # causal keep-mask for a diagonal [j=128, i=128] tile:
            # mask[p, f] = 1.0 if f - p >= 0 else 0.0
            maskc = consts.tile([128, 128], BF16, tag="maskc")
            if ab:
                dummy_f = consts.tile([128, 1024], F32, tag="dummy_f")
                nc.gpsimd.memset(dummy_f[:], 1.0)
                dummy_b = consts.tile([128, 2048], BF16, tag="dummy_b")
                nc.gpsimd.memset(dummy_b[:], 1.0)
            nc.gpsimd.memset(maskc[:], 1.0)
            nc.gpsimd.affine_select(
                out=maskc[:],
                in_=maskc[:],
                compare_op=mybir.AluOpType.is_ge,
                fill=0.0,
                base=0,
                channel_multiplier=-1,
                pattern=[[1, 128]],
            )

            # ---- input DMAs ----
            XT = []
            for c in range(N_CT):
                t_ = p_xt.tile([128, T], BF16, tag=f"xt{c}")
                nc.sync.dma_start(t_[:], xt[c * 128 : (c + 1) * 128, :])
                XT.append(t_)
            WT = []
            for c in range(N_CT):
                t_ = p_wt.tile([128, 3 * FQK], BF16, tag=f"wt{c}")
                nc.sync.dma_start(t_[:], wt[c * 128 : (c + 1) * 128, :])
                WT.append(t_)
            WB = p_wt.tile([1, 3 * FQK], BF16, tag="wb")
            nc.sync.dma_start(WB[:], wb[:, :])
            WP = []
            for d in range(4):
                t_ = p_wp.tile([128, C], BF16, tag=f"wp{d}")
                nc.sync.dma_start(t_[:], wp[d * 128 : (d + 1) * 128, :])
                WP.append(t_)

            # ---- V natural [t, 512] -> V_ext [t, 8*65 (+pad)] (ones col per
            # head; padded so every head has a 128-wide lhsT window for FWL) ----
            VEXW = HPC * VE + 128 - VE  # 583 -> pad a bit for alignment
            VEX = []
            for tt in range(N_TT if use_qkv else 0):
                vex = p_vex.tile([128, VEXW], BF16, tag=f"vex{tt}")
                nc.gpsimd.memset(vex[:], 1.0)
                ps = ps_qkv.tile([128, 512], F32, tag="qkv")
                for c in range(N_CT):
                    nc.tensor.matmul(
                        ps[:],
                        lhsT=XT[c][:, tt * 128 : (tt + 1) * 128],
                        rhs=WT[c][:, 2 * FQK : 3 * FQK],
                        start=(c == 0),
                        stop=False,
                    )
                nc.tensor.matmul(
                    ps[:],
                    lhsT=ones[0:1, 0:128],
                    rhs=WB[0:1, 2 * FQK : 3 * FQK],
                    start=False,
                    stop=True,
                )
                dst = vex[:, 0 : HPC * VE].rearrange("p (h e) -> p h e", h=HPC)[
                    :, :, 0:HD
                ]
                src = ps[:].rearrange("p (h e) -> p h e", h=HPC)
                nc.vector.tensor_copy(dst, src)
                VEX.append(vex)

            # ---- per head pair: QKV^T tiles then attention ----
            QKT = [None] * 8  # f-tiles: 0-3 = Q features, 4-7 = K features
            YT = []
            for hp in range(N_HP):
                if use_pv:
                    yt = p_yt.tile([128, T], BF16, tag=f"yt{hp}")
                else:
                    yt = dummy_b
                YT.append(yt)

            # QKV^T feature-major: out[f, t] = sum_c W[c, f] * XT[c, t]
            for ft in range(8) if use_qkv else ():
                qkt = p_qkt.tile([128, T], BF16, tag=f"qkt{ft}")
                QKT[ft] = qkt
                for tb in range(N_TB):
                    ps = ps_qkv.tile([128, 512], F32, tag="qkv")
                    for c in range(N_CT):
                        nc.tensor.matmul(
                            ps[:],
                            lhsT=WT[c][:, ft * 128 : (ft + 1) * 128],
                            rhs=XT[c][:, tb * 512 : (tb + 1) * 512],
                            start=(c == 0),
                            stop=False,
                        )
                    nc.tensor.matmul(
                        ps[:],
                        lhsT=WB[0:1, ft * 128 : (ft + 1) * 128],
                        rhs=ones[0:1, 0:512],
                        start=False,
                        stop=True,
                    )
                    nc.vector.tensor_copy(qkt[:, tb * 512 : (tb + 1) * 512], ps[:])

            # ---- attention: flat (hp, ib, jt) stream, S emitted one triple
            # ahead of exp/PV so the PE never queues behind the ACT exp ----
            triples = []
            for hp in range(N_HP):
                for ib in range(N_IB):
                    for jt in range(4 * (ib + 1)):
                        triples.append((hp, ib, jt))
            pvs = {}

            def emit_s(t):
                hp, ib, jt = t
                i_base, j_base = 512 * ib, 128 * jt
                i_lo = max(i_base, j_base)
                w = i_base + 512 - i_lo
                rel = i_lo - i_base
                if not use_s:
                    return None
                qt, kt = QKT[hp], QKT[4 + hp]
                ps = ps_s.tile([128, 1024], F32, tag="s", name="ps")
                # S^T[j, i] for heads A (rows 0:64 of qkt/kt) and B
                nc.tensor.matmul(
                    ps[:, rel : rel + w],
                    lhsT=kt[0:64, j_base : j_base + 128],
                    rhs=qt[0:64, i_lo : i_lo + w],
                    start=True,
                    stop=True,
                )
                nc.tensor.matmul(
                    ps[:, 512 + rel : 512 + rel + w],
                    lhsT=kt[64:128, j_base : j_base + 128],
                    rhs=qt[64:128, i_lo : i_lo + w],
                    start=True,
                    stop=True,
                )
                return ps

            def emit_rest(t, ps):
                hp, ib, jt = t
                i_base, j_base = 512 * ib, 128 * jt
                njt = 4 * (ib + 1)
                i_lo = max(i_base, j_base)
                w = i_base + 512 - i_lo
                rel = i_lo - i_base
                ex = None
                if use_exp:
                    ex = p_ex.tile([128, 1024], BF16, tag="ex", name="ex")
                    src = ps if use_s else dummy_f
                    ps3 = src[:].rearrange("p (b q) -> p b q", b=2)[
                        :, :, rel : rel + w
                    ]
                    ex3 = ex[:].rearrange("p (b q) -> p b q", b=2)[
                        :, :, rel : rel + w
                    ]
                    if "expcopy" in ab:
                        nc.vector.tensor_copy(ex3, ps3)
                    else:
                        nc.scalar.activation(
                            ex3, ps3, mybir.ActivationFunctionType.Exp, scale=SCALE
                        )
                    if j_base >= i_base and use_mask:
                        # diagonal tile: zero out j > i in [i_lo, i_lo+128)
                        # (on GpSimd: DVE is the busier engine)
                        for h2 in range(2):
                            sl = ex[:, 512 * h2 + rel : 512 * h2 + rel + 128]
                            nc.gpsimd.tensor_mul(sl, sl, maskc[:])
                if not use_pv:
                    return
                if jt == 0:
                    pvA = ps_pv.tile([128, 512], F32, tag="pvA", name="pvA")
                    pvB = ps_pv.tile([128, 512], F32, tag="pvB", name="pvB")
                    pvs[(hp, ib)] = (pvA, pvB)
                exsrc = ex if use_exp else dummy_b
                for h2, pv in ((0, pvs[(hp, ib)][0]), (1, pvs[(hp, ib)][1])):
                    # 128-wide stationary window (cols >= 65 produce garbage
                    # rows 65.. of pv that normalize ignores) -> FWL stays on
                    off = (2 * hp + h2) * VE
                    nc.tensor.matmul(
                        pv[:, rel : rel + w],
                        lhsT=VEX[jt][:, off : off + 128],
                        rhs=exsrc[:, 512 * h2 + rel : 512 * h2 + rel + w],
                        start=(jt == 0),
                        stop=(jt == njt - 1),
                        skip_group_check=True,
                    )
                if jt == njt - 1:
                    # normalize: rows 0:64 divided by row 64 (ones-column sum)
                    for h2, pv in ((0, pvs[(hp, ib)][0]), (1, pvs[(hp, ib)][1])):
                        rsb = p_small.tile([1, 512], BF16, tag="rsb")
                        with nc.allow_low_precision(reason="softmax denom bcast"):
                            nc.vector.reciprocal(rsb[:], pv[HD : HD + 1, :])
                        bc = ps_qkv.tile([64, 512], F32, tag="qkv", name="bc")
                        nc.tensor.matmul(
                            bc[:],
                            lhsT=ones[0:1, 0:64],
                            rhs=rsb[:],
                            start=True,
                            stop=True,
                        )
                        bcs = p_small.tile([64, 512], BF16, tag="bcs")
                        nc.vector.tensor_copy(bcs[:], bc[:])
                        nc.vector.tensor_mul(
                            YT[hp][
                                64 * h2 : 64 * h2 + 64,
                                i_base : i_base + 512,
                            ],
                            pv[0:HD, :],
                            bcs[:],
                        )

            prev = None
            prev_ps = None
            for t in triples:
                cur_ps = emit_s(t)
                if prev is not None:
                    emit_rest(prev, prev_ps)
                prev, prev_ps = t, cur_ps
            if prev is not None:
                emit_rest(prev, prev_ps)

            # ---- out^T = wp^T @ YT  (co-major) ----
            for ct in range(8):
                stage = p_stage.tile([128, T], F32, tag="st")
                for tb in range(N_TB):
                    if use_proj:
                        ps = ps_qkv.tile([128, 512], F32, tag="qkv")
                        for d in range(4):
                            nc.tensor.matmul(
                                ps[:],
                                lhsT=WP[d][:, ct * 128 : (ct + 1) * 128],
                                rhs=YT[d][:, tb * 512 : (tb + 1) * 512],
                                start=(d == 0),
                                stop=(d == 3),
                            )
                        nc.vector.tensor_copy(
                            stage[:, tb * 512 : (tb + 1) * 512], ps[:]
                        )
                    else:
                        nc.vector.tensor_copy(
                            stage[:, tb * 512 : (tb + 1) * 512], dummy_f[:, 0:512]
                        )
                nc.sync.dma_start(outT[ct * 128 : (ct + 1) * 128, :], stage[:])

    nc.compile()
    return nc


_NC = None


def _get_nc():
    global _NC
    if _NC is None:
        _NC = build_nc()
    return _NC


def make_in_maps(x, W_attn, b_attn, W_proj):
    bf = ml_dtypes.bfloat16
    xt_b = [np.ascontiguousarray(x[b].astype(bf).T) for b in range(B)]
    in_maps = []
    for core in range(8):
        b, g = core // 2, core % 2
        wq = W_attn[:, 512 * g : 512 * g + 512]
        wk = W_attn[:, C + 512 * g : C + 512 * g + 512]
        wv = W_attn[:, 2 * C + 512 * g : 2 * C + 512 * g + 512]
        wt = np.ascontiguousarray(np.concatenate([wq, wk, wv], axis=1)).astype(bf)
        bq = b_attn[512 * g : 512 * g + 512]
        bk = b_attn[C + 512 * g : C + 512 * g + 512]
        bv = b_attn[2 * C + 512 * g : 2 * C + 512 * g + 512]
        wb_ = np.concatenate([bq, bk, bv])[None, :].astype(bf)
        wp_ = np.ascontiguousarray(W_proj[512 * g : 512 * g + 512, :]).astype(bf)
        in_maps.append({"xt": xt_b[b], "wt": wt, "wb": wb_, "wp": wp_})
    return in_maps


def combine_outputs(results, b_proj):
    out = np.empty((B, T, C), dtype=np.float32)
    for b in range(B):
        acc = results[2 * b]["outT"] + results[2 * b + 1]["outT"]
        out[b] = acc.T + b_proj[None, :]
    return out


def kernel(x, W_attn, b_attn, W_proj, b_proj):
    x = np.asarray(x, dtype=np.float32)
    W_attn = np.asarray(W_attn, dtype=np.float32)
    b_attn = np.asarray(b_attn, dtype=np.float32)
    W_proj = np.asarray(W_proj, dtype=np.float32)
    b_proj = np.asarray(b_proj, dtype=np.float32)

    nc = _get_nc()
    in_maps = make_in_maps(x, W_attn, b_attn, W_proj)
    res = run_bass_kernel_spmd(nc, in_maps, core_ids=list(range(8)))
    return combine_outputs(res.results, b_proj)


# revision 33
# speedup vs baseline: 2.4787x; 1.1948x over previous
"""Causal self-attention block (B=4, T=2048, C=1024, H=16) on 8 TRN2 NeuronCores.

Sharding: core -> (batch b = core//2, head-group g = core%2, 8 heads each).
Each core computes, for its (b, g):
    qkv slice -> per-head causal softmax(Q K^T * scale) V -> partial out^T =
    W_proj[g-rows]^T @ Y^T  (f32 [1024, 2048])
Host sums the two group partials per batch, transposes, adds b_proj.

All matmuls in bf16 (fp32 accumulation in PSUM).
"""

import numpy as np
import ml_dtypes

import concourse.bass as bass
import concourse.mybir as mybir
import concourse.tile as tile
from concourse import bacc
from concourse.bass_utils import run_bass_kernel_spmd

BF16 = mybir.dt.bfloat16
F32 = mybir.dt.float32

B, T, C = 4, 2048, 1024
H = 16          # total heads
HD = 64         # head dim
HPC = 8         # heads per core
G = 2           # head groups (cores per batch)
FQK = 512       # q (or k) features per core
FV = 512        # v features per core
VE = 65         # v features + ones column, per head
SCALE = float(0.1 / np.sqrt(np.float32(HD)))

N_CT = 8        # c chunks of 128
N_TT = 16       # t tiles of 128
N_TB = 4        # t blocks of 512
N_HP = 4        # head pairs per core
N_IB = 4        # i blocks of 512


def build_nc(loop_n: int = 1, ablate: str = "", with_bias: bool = True) -> bacc.Bacc:
    """ablate: comma tokens from {exp, s, pv, attn, mask} to skip stages (timing only)."""
    ab = set(t for t in ablate.split(",") if t)
    use_s = "s" not in ab and "attn" not in ab
    use_exp = "exp" not in ab and "attn" not in ab
    use_mask = "mask" not in ab and "attn" not in ab
    use_pv = "pv" not in ab and "attn" not in ab
    use_qkv = "qkv" not in ab
    use_proj = "proj" not in ab
    if not use_qkv:
        assert "attn" in ab, "qkv ablation requires attn ablation"
    nc = bacc.Bacc("TRN2", target_bir_lowering=False, debug=False, num_devices=8)

    xt = nc.dram_tensor("xt", [C, T], BF16, kind="ExternalInput")
    wt = nc.dram_tensor("wt", [C, 3 * FQK], BF16, kind="ExternalInput")
    wb = nc.dram_tensor("wb", [1, 3 * FQK], BF16, kind="ExternalInput")
    wp = nc.dram_tensor("wp", [FV, C], BF16, kind="ExternalInput")
    outT = nc.dram_tensor("outT", [C, T], F32, kind="ExternalOutput")

    with tile.TileContext(nc) as tc:
        from contextlib import ExitStack

        with ExitStack() as ctx:
            if loop_n > 1:
                ctx.enter_context(tc.For_i(0, loop_n, 1))
            consts = ctx.enter_context(tc.tile_pool(name="consts", bufs=1))
            p_xt = ctx.enter_context(tc.tile_pool(name="p_xt", bufs=1))
            p_wt = ctx.enter_context(tc.tile_pool(name="p_wt", bufs=1))
            p_wp = ctx.enter_context(tc.tile_pool(name="p_wp", bufs=1))
            p_qkt = ctx.enter_context(tc.tile_pool(name="p_qkt", bufs=1))
            p_vex = ctx.enter_context(tc.tile_pool(name="p_vex", bufs=1))
            p_yt = ctx.enter_context(tc.tile_pool(name="p_yt", bufs=1))
            p_ex = ctx.enter_context(tc.tile_pool(name="p_ex", bufs=4))
            p_small = ctx.enter_context(tc.tile_pool(name="p_small", bufs=4))
            p_stage = ctx.enter_context(tc.tile_pool(name="p_stage", bufs=2))
            ps_qkv = ctx.enter_context(
                tc.tile_pool(name="ps_qkv", bufs=2, space="PSUM")
            )
            ps_s = ctx.enter_context(tc.tile_pool(name="ps_s", bufs=2, space="PSUM"))
            ps_pv = ctx.enter_context(tc.tile_pool(name="ps_pv", bufs=1, space="PSUM"))

            # ---- constants ----
            ones = consts.tile([1, 512], BF16, tag="ones")
            nc.gpsimd.memset(ones[:], 1.0)
            # causal keep-mask for a diagonal [j=128, i=128] tile:
            # mask[p, f] = 1.0 if f - p >= 0 else 0.0
            maskc = consts.tile([128, 128], BF16, tag="maskc")
            if ab:
                dummy_f = consts.tile([128, 1024], F32, tag="dummy_f")
                nc.gpsimd.memset(dummy_f[:], 1.0)
                dummy_b = consts.tile([128, 2048], BF16, tag="dummy_b")
                nc.gpsimd.memset(dummy_b[:], 1.0)
            nc.gpsimd.memset(maskc[:], 1.0)
            nc.gpsimd.affine_select(
                out=maskc[:],
                in_=maskc[:],
                compare_op=mybir.AluOpType.is_ge,
                fill=0.0,
                base=0,
                channel_multiplier=-1,
                pattern=[[1, 128]],
            )

            # ---- input DMAs ----
            XT = []
            for c in range(N_CT):
                t_ = p_xt.tile([128, T], BF16, tag=f"xt{c}")
                nc.sync.dma_start(t_[:], xt[c * 128 : (c + 1) * 128, :])
                XT.append(t_)
            WT = []
            for c in range(N_CT):
                t_ = p_wt.tile([128, 3 * FQK], BF16, tag=f"wt{c}")
                nc.sync.dma_start(t_[:], wt[c * 128 : (c + 1) * 128, :])
                WT.append(t_)
            WB = p_wt.tile([1, 3 * FQK], BF16, tag="wb")
            nc.sync.dma_start(WB[:], wb[:, :])
            WP = []
            for d in range(4):
                t_ = p_wp.tile([128, C], BF16, tag=f"wp{d}")
                nc.sync.dma_start(t_[:], wp[d * 128 : (d + 1) * 128, :])
                WP.append(t_)

            # ---- V natural [t, 512] -> V_ext [t, 8*65 (+pad)] (ones col per
            # head; padded so every head has a 128-wide lhsT window for FWL) ----
            VEXW = HPC * VE + 128 - VE  # 583 -> pad a bit for alignment
            VEX = []
            for tt in range(N_TT if use_qkv else 0):
                vex = p_vex.tile([128, VEXW], BF16, tag=f"vex{tt}")
                nc.gpsimd.memset(vex[:], 1.0)
                ps = ps_qkv.tile([128, 512], F32, tag="qkv")
                for c in range(N_CT):
                    nc.tensor.matmul(
                        ps[:],
                        lhsT=XT[c][:, tt * 128 : (tt + 1) * 128],
                        rhs=WT[c][:, 2 * FQK : 3 * FQK],
                        start=(c == 0),
                        stop=(not with_bias and c == N_CT - 1),
                    )
                if with_bias:
                    nc.tensor.matmul(
                        ps[:],
                        lhsT=ones[0:1, 0:128],
                        rhs=WB[0:1, 2 * FQK : 3 * FQK],
                        start=False,
                        stop=True,
                    )
                dst = vex[:, 0 : HPC * VE].rearrange("p (h e) -> p h e", h=HPC)[
                    :, :, 0:HD
                ]
                src = ps[:].rearrange("p (h e) -> p h e", h=HPC)
                nc.vector.tensor_copy(dst, src)
                VEX.append(vex)

            # ---- per head pair: QKV^T tiles then attention ----
            QKT = [None] * 8  # f-tiles: 0-3 = Q features, 4-7 = K features
            YT = []
            for hp in range(N_HP):
                if use_pv:
                    yt = p_yt.tile([128, T], BF16, tag=f"yt{hp}")
                else:
                    yt = dummy_b
                YT.append(yt)

            # QKV^T feature-major: out[f, t] = sum_c W[c, f] * XT[c, t]
            for ft in range(8) if use_qkv else ():
                qkt = p_qkt.tile([128, T], BF16, tag=f"qkt{ft}")
                QKT[ft] = qkt
                for tb in range(N_TB):
                    ps = ps_qkv.tile([128, 512], F32, tag="qkv")
                    for c in range(N_CT):
                        nc.tensor.matmul(
                            ps[:],
                            lhsT=WT[c][:, ft * 128 : (ft + 1) * 128],
                            rhs=XT[c][:, tb * 512 : (tb + 1) * 512],
                            start=(c == 0),
                            stop=(not with_bias and c == N_CT - 1),
                        )
                    if with_bias:
                        nc.tensor.matmul(
                            ps[:],
                            lhsT=WB[0:1, ft * 128 : (ft + 1) * 128],
                            rhs=ones[0:1, 0:512],
                            start=False,
                            stop=True,
                        )
                    nc.vector.tensor_copy(qkt[:, tb * 512 : (tb + 1) * 512], ps[:])

            # ---- attention: flat (hp, ib, jt) stream, S emitted one triple
            # ahead of exp/PV so the PE never queues behind the ACT exp ----
            triples = []
            for hp in range(N_HP):
                for ib in range(N_IB):
                    for jt in range(4 * (ib + 1)):
                        triples.append((hp, ib, jt))
            pvs = {}

            def emit_s(t):
                hp, ib, jt = t
                i_base, j_base = 512 * ib, 128 * jt
                i_lo = max(i_base, j_base)
                w = i_base + 512 - i_lo
                rel = i_lo - i_base
                if not use_s:
                    return None
                qt, kt = QKT[hp], QKT[4 + hp]
                ps = ps_s.tile([128, 1024], F32, tag="s", name="ps")
                # S^T[j, i] for heads A (rows 0:64 of qkt/kt) and B
                nc.tensor.matmul(
                    ps[:, rel : rel + w],
                    lhsT=kt[0:64, j_base : j_base + 128],
                    rhs=qt[0:64, i_lo : i_lo + w],
                    start=True,
                    stop=True,
                )
                nc.tensor.matmul(
                    ps[:, 512 + rel : 512 + rel + w],
                    lhsT=kt[64:128, j_base : j_base + 128],
                    rhs=qt[64:128, i_lo : i_lo + w],
                    start=True,
                    stop=True,
                )
                return ps

            def emit_rest(t, ps):
                hp, ib, jt = t
                i_base, j_base = 512 * ib, 128 * jt
                njt = 4 * (ib + 1)
                i_lo = max(i_base, j_base)
                w = i_base + 512 - i_lo
                rel = i_lo - i_base
                ex = None
                if use_exp:
                    ex = p_ex.tile([128, 1024], BF16, tag="ex", name="ex")
                    src = ps if use_s else dummy_f
                    ps3 = src[:].rearrange("p (b q) -> p b q", b=2)[
                        :, :, rel : rel + w
                    ]
                    ex3 = ex[:].rearrange("p (b q) -> p b q", b=2)[
                        :, :, rel : rel + w
                    ]
                    if "expcopy" in ab:
                        nc.vector.tensor_copy(ex3, ps3)
                    else:
                        nc.scalar.activation(
                            ex3, ps3, mybir.ActivationFunctionType.Exp, scale=SCALE
                        )
                    if j_base >= i_base and use_mask:
                        # diagonal tile: zero out j > i in [i_lo, i_lo+128)
                        # (on GpSimd: DVE is the busier engine)
                        for h2 in range(2):
                            sl = ex[:, 512 * h2 + rel : 512 * h2 + rel + 128]
                            nc.gpsimd.tensor_mul(sl, sl, maskc[:])
                if not use_pv:
                    return
                if jt == 0:
                    pvA = ps_pv.tile([128, 512], F32, tag="pvA", name="pvA")
                    pvB = ps_pv.tile([128, 512], F32, tag="pvB", name="pvB")
                    pvs[(hp, ib)] = (pvA, pvB)
                exsrc = ex if use_exp else dummy_b
                for h2, pv in ((0, pvs[(hp, ib)][0]), (1, pvs[(hp, ib)][1])):
                    # 128-wide stationary window (cols >= 65 produce garbage
                    # rows 65.. of pv that normalize ignores) -> FWL stays on
                    off = (2 * hp + h2) * VE
                    nc.tensor.matmul(
                        pv[:, rel : rel + w],
                        lhsT=VEX[jt][:, off : off + 128],
                        rhs=exsrc[:, 512 * h2 + rel : 512 * h2 + rel + w],
                        start=(jt == 0),
                        stop=(jt == njt - 1),
                        skip_group_check=True,
                    )
                if jt == njt - 1:
                    # normalize: rows 0:64 divided by row 64 (ones-column sum)
                    for h2, pv in ((0, pvs[(hp, ib)][0]), (1, pvs[(hp, ib)][1])):
                        rsb = p_small.tile([1, 512], BF16, tag="rsb")
                        with nc.allow_low_precision(reason="softmax denom bcast"):
                            nc.vector.reciprocal(rsb[:], pv[HD : HD + 1, :])
                        bc = ps_qkv.tile([64, 512], F32, tag="qkv", name="bc")
                        nc.tensor.matmul(
                            bc[:],
                            lhsT=ones[0:1, 0:64],
                            rhs=rsb[:],
                            start=True,
                            stop=True,
                        )
                        bcs = p_small.tile([64, 512], BF16, tag="bcs")
                        nc.vector.tensor_copy(bcs[:], bc[:])
                        nc.vector.tensor_mul(
                            YT[hp][
                                64 * h2 : 64 * h2 + 64,
                                i_base : i_base + 512,
                            ],
                            pv[0:HD, :],
                            bcs[:],
                        )

            prev = None
            prev_ps = None
            for t in triples:
                cur_ps = emit_s(t)
                if prev is not None:
                    emit_rest(prev, prev_ps)
                prev, prev_ps = t, cur_ps
            if prev is not None:
                emit_rest(prev, prev_ps)

            # ---- out^T = wp^T @ YT  (co-major) ----
            for ct in range(8):
                stage = p_stage.tile([128, T], F32, tag="st")
                for tb in range(N_TB):
                    if use_proj:
                        ps = ps_qkv.tile([128, 512], F32, tag="qkv")
                        for d in range(4):
                            nc.tensor.matmul(
                                ps[:],
                                lhsT=WP[d][:, ct * 128 : (ct + 1) * 128],
                                rhs=YT[d][:, tb * 512 : (tb + 1) * 512],
                                start=(d == 0),
                                stop=(d == 3),
                            )
                        nc.vector.tensor_copy(
                            stage[:, tb * 512 : (tb + 1) * 512], ps[:]
                        )
                    else:
                        nc.vector.tensor_copy(
                            stage[:, tb * 512 : (tb + 1) * 512], dummy_f[:, 0:512]
                        )
                nc.sync.dma_start(outT[ct * 128 : (ct + 1) * 128, :], stage[:])

    nc.compile()
    return nc


_NC = {}


def _get_nc(with_bias: bool = True):
    key = bool(with_bias)
    if key not in _NC:
        _NC[key] = build_nc(with_bias=key)
    return _NC[key]


def make_in_maps(x, W_attn, b_attn, W_proj):
    bf = ml_dtypes.bfloat16
    xt_b = [np.ascontiguousarray(x[b].astype(bf).T) for b in range(B)]
    in_maps = []
    for core in range(8):
        b, g = core // 2, core % 2
        wq = W_attn[:, 512 * g : 512 * g + 512]
        wk = W_attn[:, C + 512 * g : C + 512 * g + 512]
        wv = W_attn[:, 2 * C + 512 * g : 2 * C + 512 * g + 512]
        wt = np.ascontiguousarray(np.concatenate([wq, wk, wv], axis=1)).astype(bf)
        bq = b_attn[512 * g : 512 * g + 512]
        bk = b_attn[C + 512 * g : C + 512 * g + 512]
        bv = b_attn[2 * C + 512 * g : 2 * C + 512 * g + 512]
        wb_ = np.concatenate([bq, bk, bv])[None, :].astype(bf)
        wp_ = np.ascontiguousarray(W_proj[512 * g : 512 * g + 512, :]).astype(bf)
        in_maps.append({"xt": xt_b[b], "wt": wt, "wb": wb_, "wp": wp_})
    return in_maps


def combine_outputs(results, b_proj):
    out = np.empty((B, T, C), dtype=np.float32)
    for b in range(B):
        acc = results[2 * b]["outT"] + results[2 * b + 1]["outT"]
        out[b] = acc.T + b_proj[None, :]
    return out


def kernel(x, W_attn, b_attn, W_proj, b_proj):
    x = np.asarray(x, dtype=np.float32)
    W_attn = np.asarray(W_attn, dtype=np.float32)
    b_attn = np.asarray(b_attn, dtype=np.float32)
    W_proj = np.asarray(W_proj, dtype=np.float32)
    b_proj = np.asarray(b_proj, dtype=np.float32)

    nc = _get_nc(with_bias=True)
    in_maps = make_in_maps(x, W_attn, b_attn, W_proj)
    res = run_bass_kernel_spmd(nc, in_maps, core_ids=list(range(8)))
    return combine_outputs(res.results, b_proj)


# revision 40
# speedup vs baseline: 2.5662x; 1.0353x over previous
"""Causal self-attention block (B=4, T=2048, C=1024, H=16) on 8 TRN2 NeuronCores.

Sharding: core -> (batch b = core//2, head-group g = core%2, 8 heads each).
Each core computes, for its (b, g):
    qkv slice -> per-head causal softmax(Q K^T * scale) V -> partial out^T =
    W_proj[g-rows]^T @ Y^T  (f32 [1024, 2048])
Host sums the two group partials per batch, transposes, adds b_proj.

All matmuls in bf16 (fp32 accumulation in PSUM).
"""

import numpy as np
import ml_dtypes

import concourse.bass as bass
import concourse.mybir as mybir
import concourse.tile as tile
from concourse import bacc
from concourse.bass_utils import run_bass_kernel_spmd

BF16 = mybir.dt.bfloat16
F32 = mybir.dt.float32

B, T, C = 4, 2048, 1024
H = 16          # total heads
HD = 64         # head dim
HPC = 8         # heads per core
G = 2           # head groups (cores per batch)
FQK = 512       # q (or k) features per core
FV = 512        # v features per core
VE = 65         # v features + ones column, per head
SCALE = float(0.1 / np.sqrt(np.float32(HD)))

N_CT = 8        # c chunks of 128
N_TT = 16       # t tiles of 128
N_TB = 4        # t blocks of 512
N_HP = 4        # head pairs per core
N_IB = 4        # i blocks of 512


def build_nc(
    loop_n: int = 1,
    ablate: str = "",
    with_bias: bool = True,
    mask_on_gpsimd: bool = False,
) -> bacc.Bacc:
    """ablate: comma tokens from {exp, s, pv, attn, mask} to skip stages (timing only)."""
    ab = set(t for t in ablate.split(",") if t)
    use_s = "s" not in ab and "attn" not in ab
    use_exp = "exp" not in ab and "attn" not in ab
    use_mask = "mask" not in ab and "attn" not in ab
    use_pv = "pv" not in ab and "attn" not in ab
    use_qkv = "qkv" not in ab
    use_proj = "proj" not in ab
    if not use_qkv:
        assert "attn" in ab, "qkv ablation requires attn ablation"
    nc = bacc.Bacc("TRN2", target_bir_lowering=False, debug=False, num_devices=8)

    xt = nc.dram_tensor("xt", [C, T], BF16, kind="ExternalInput")
    wt = nc.dram_tensor("wt", [C, 3 * FQK], BF16, kind="ExternalInput")
    wb = nc.dram_tensor("wb", [1, 3 * FQK], BF16, kind="ExternalInput")
    wp = nc.dram_tensor("wp", [FV, C], BF16, kind="ExternalInput")
    outT = nc.dram_tensor("outT", [C, T], F32, kind="ExternalOutput")

    with tile.TileContext(nc) as tc:
        from contextlib import ExitStack

        with ExitStack() as ctx:
            if loop_n > 1:
                ctx.enter_context(tc.For_i(0, loop_n, 1))
            consts = ctx.enter_context(tc.tile_pool(name="consts", bufs=1))
            p_xt = ctx.enter_context(tc.tile_pool(name="p_xt", bufs=1))
            p_wt = ctx.enter_context(tc.tile_pool(name="p_wt", bufs=1))
            p_wp = ctx.enter_context(tc.tile_pool(name="p_wp", bufs=1))
            p_qkt = ctx.enter_context(tc.tile_pool(name="p_qkt", bufs=1))
            p_vex = ctx.enter_context(tc.tile_pool(name="p_vex", bufs=1))
            p_yt = ctx.enter_context(tc.tile_pool(name="p_yt", bufs=1))
            p_ex = ctx.enter_context(tc.tile_pool(name="p_ex", bufs=4))
            p_small = ctx.enter_context(tc.tile_pool(name="p_small", bufs=4))
            p_stage = ctx.enter_context(tc.tile_pool(name="p_stage", bufs=3))
            ps_qkv = ctx.enter_context(
                tc.tile_pool(name="ps_qkv", bufs=2, space="PSUM")
            )
            ps_s = ctx.enter_context(tc.tile_pool(name="ps_s", bufs=2, space="PSUM"))
            ps_pv = ctx.enter_context(tc.tile_pool(name="ps_pv", bufs=1, space="PSUM"))

            # ---- constants ----
            ones = consts.tile([1, 512], BF16, tag="ones")
            nc.gpsimd.memset(ones[:], 1.0)
            # causal keep-mask for a diagonal [j=128, i=128] tile:
            # mask[p, f] = 1.0 if f - p >= 0 else 0.0
            maskc = consts.tile([128, 128], BF16, tag="maskc")
            if ab:
                dummy_f = consts.tile([128, 1024], F32, tag="dummy_f")
                nc.gpsimd.memset(dummy_f[:], 1.0)
                dummy_b = consts.tile([128, 2048], BF16, tag="dummy_b")
                nc.gpsimd.memset(dummy_b[:], 1.0)
            nc.gpsimd.memset(maskc[:], 1.0)
            nc.gpsimd.affine_select(
                out=maskc[:],
                in_=maskc[:],
                compare_op=mybir.AluOpType.is_ge,
                fill=0.0,
                base=0,
                channel_multiplier=-1,
                pattern=[[1, 128]],
            )

            # ---- input DMAs ----
            XT = []
            for c in range(N_CT):
                t_ = p_xt.tile([128, T], BF16, tag=f"xt{c}")
                nc.sync.dma_start(t_[:], xt[c * 128 : (c + 1) * 128, :])
                XT.append(t_)
            WT = []
            for c in range(N_CT):
                t_ = p_wt.tile([128, 3 * FQK], BF16, tag=f"wt{c}")
                nc.sync.dma_start(t_[:], wt[c * 128 : (c + 1) * 128, :])
                WT.append(t_)
            WB = p_wt.tile([1, 3 * FQK], BF16, tag="wb")
            nc.sync.dma_start(WB[:], wb[:, :])
            WP = []
            for d in range(4):
                t_ = p_wp.tile([128, C], BF16, tag=f"wp{d}")
                nc.sync.dma_start(t_[:], wp[d * 128 : (d + 1) * 128, :])
                WP.append(t_)

            # ---- V natural [t, 512] -> V_ext [t, 8*65 (+pad)] (ones col per
            # head; padded so every head has a 128-wide lhsT window for FWL) ----
            VEXW = HPC * VE + 128 - VE  # 583 -> pad a bit for alignment
            VEX = []
            for tt in range(N_TT if use_qkv else 0):
                vex = p_vex.tile([128, VEXW], BF16, tag=f"vex{tt}")
                nc.gpsimd.memset(vex[:], 1.0)
                ps = ps_qkv.tile([128, 512], F32, tag="qkv")
                for c in range(N_CT):
                    nc.tensor.matmul(
                        ps[:],
                        lhsT=XT[c][:, tt * 128 : (tt + 1) * 128],
                        rhs=WT[c][:, 2 * FQK : 3 * FQK],
                        start=(c == 0),
                        stop=(not with_bias and c == N_CT - 1),
                    )
                if with_bias:
                    nc.tensor.matmul(
                        ps[:],
                        lhsT=ones[0:1, 0:128],
                        rhs=WB[0:1, 2 * FQK : 3 * FQK],
                        start=False,
                        stop=True,
                    )
                dst = vex[:, 0 : HPC * VE].rearrange("p (h e) -> p h e", h=HPC)[
                    :, :, 0:HD
                ]
                src = ps[:].rearrange("p (h e) -> p h e", h=HPC)
                nc.vector.tensor_copy(dst, src)
                VEX.append(vex)

            # ---- per head pair: QKV^T tiles then attention ----
            QKT = [None] * 8  # f-tiles: 0-3 = Q features, 4-7 = K features
            YT = []
            for hp in range(N_HP):
                if use_pv:
                    yt = p_yt.tile([128, T], BF16, tag=f"yt{hp}")
                else:
                    yt = dummy_b
                YT.append(yt)

            # QKV^T feature-major: out[f, t] = sum_c W[c, f] * XT[c, t]
            for ft in range(8) if use_qkv else ():
                qkt = p_qkt.tile([128, T], BF16, tag=f"qkt{ft}")
                QKT[ft] = qkt
                for tb in range(N_TB):
                    ps = ps_qkv.tile([128, 512], F32, tag="qkv")
                    for c in range(N_CT):
                        nc.tensor.matmul(
                            ps[:],
                            lhsT=WT[c][:, ft * 128 : (ft + 1) * 128],
                            rhs=XT[c][:, tb * 512 : (tb + 1) * 512],
                            start=(c == 0),
                            stop=(not with_bias and c == N_CT - 1),
                        )
                    if with_bias:
                        nc.tensor.matmul(
                            ps[:],
                            lhsT=WB[0:1, ft * 128 : (ft + 1) * 128],
                            rhs=ones[0:1, 0:512],
                            start=False,
                            stop=True,
                        )
                    nc.vector.tensor_copy(qkt[:, tb * 512 : (tb + 1) * 512], ps[:])

            # ---- attention: flat (hp, ib, jt) stream, S emitted one triple
            # ahead of exp/PV so the PE never queues behind the ACT exp.
            # ib-major order: each i-block's YT finishes across all head
            # pairs early, so that t-block's projection can interleave. ----
            triples = []
            for ib in range(N_IB):
                for hp in range(N_HP):
                    for jt in range(4 * (ib + 1)):
                        triples.append((hp, ib, jt))
            pvs = {}

            def emit_proj(tb):
                # out^T[:, tb-block] = wp^T @ YT[:, tb-block]
                for ct in range(8):
                    if use_proj:
                        psp = ps_qkv.tile([128, 512], F32, tag="qkv", name="psp")
                        for d in range(4):
                            nc.tensor.matmul(
                                psp[:],
                                lhsT=WP[d][:, ct * 128 : (ct + 1) * 128],
                                rhs=YT[d][:, tb * 512 : (tb + 1) * 512],
                                start=(d == 0),
                                stop=(d == 3),
                            )
                        stage = p_stage.tile([128, 512], F32, tag="st")
                        nc.vector.tensor_copy(stage[:], psp[:])
                    else:
                        stage = p_stage.tile([128, 512], F32, tag="st")
                        nc.vector.tensor_copy(stage[:], dummy_f[:, 0:512])
                    nc.sync.dma_start(
                        outT[ct * 128 : (ct + 1) * 128, tb * 512 : (tb + 1) * 512],
                        stage[:],
                    )

            def emit_s(t):
                hp, ib, jt = t
                i_base, j_base = 512 * ib, 128 * jt
                i_lo = max(i_base, j_base)
                w = i_base + 512 - i_lo
                rel = i_lo - i_base
                if not use_s:
                    return None
                qt, kt = QKT[hp], QKT[4 + hp]
                ps = ps_s.tile([128, 1024], F32, tag="s", name="ps")
                # S^T[j, i] for heads A (rows 0:64 of qkt/kt) and B
                nc.tensor.matmul(
                    ps[:, rel : rel + w],
                    lhsT=kt[0:64, j_base : j_base + 128],
                    rhs=qt[0:64, i_lo : i_lo + w],
                    start=True,
                    stop=True,
                )
                nc.tensor.matmul(
                    ps[:, 512 + rel : 512 + rel + w],
                    lhsT=kt[64:128, j_base : j_base + 128],
                    rhs=qt[64:128, i_lo : i_lo + w],
                    start=True,
                    stop=True,
                )
                return ps

            def emit_rest(t, ps):
                hp, ib, jt = t
                i_base, j_base = 512 * ib, 128 * jt
                njt = 4 * (ib + 1)
                i_lo = max(i_base, j_base)
                w = i_base + 512 - i_lo
                rel = i_lo - i_base
                ex = None
                if use_exp:
                    ex = p_ex.tile([128, 1024], BF16, tag="ex", name="ex")
                    src = ps if use_s else dummy_f
                    ps3 = src[:].rearrange("p (b q) -> p b q", b=2)[
                        :, :, rel : rel + w
                    ]
                    ex3 = ex[:].rearrange("p (b q) -> p b q", b=2)[
                        :, :, rel : rel + w
                    ]
                    if "expcopy" in ab:
                        nc.vector.tensor_copy(ex3, ps3)
                    else:
                        nc.scalar.activation(
                            ex3, ps3, mybir.ActivationFunctionType.Exp, scale=SCALE
                        )
                    if j_base >= i_base and use_mask:
                        # diagonal tile: zero out j > i in [i_lo, i_lo+128)
                        eng = nc.gpsimd if mask_on_gpsimd else nc.vector
                        for h2 in range(2):
                            sl = ex[:, 512 * h2 + rel : 512 * h2 + rel + 128]
                            eng.tensor_mul(sl, sl, maskc[:])
                if not use_pv:
                    return
                if jt == 0:
                    pvA = ps_pv.tile([128, 512], F32, tag="pvA", name="pvA")
                    pvB = ps_pv.tile([128, 512], F32, tag="pvB", name="pvB")
                    pvs[(hp, ib)] = (pvA, pvB)
                exsrc = ex if use_exp else dummy_b
                for h2, pv in ((0, pvs[(hp, ib)][0]), (1, pvs[(hp, ib)][1])):
                    # 128-wide stationary window (cols >= 65 produce garbage
                    # rows 65.. of pv that normalize ignores) -> FWL stays on
                    off = (2 * hp + h2) * VE
                    nc.tensor.matmul(
                        pv[:, rel : rel + w],
                        lhsT=VEX[jt][:, off : off + 128],
                        rhs=exsrc[:, 512 * h2 + rel : 512 * h2 + rel + w],
                        start=(jt == 0),
                        stop=(jt == njt - 1),
                        skip_group_check=True,
                    )
                if jt == njt - 1:
                    # normalize: rows 0:64 divided by row 64 (ones-column sum)
                    for h2, pv in ((0, pvs[(hp, ib)][0]), (1, pvs[(hp, ib)][1])):
                        rsb = p_small.tile([1, 512], BF16, tag="rsb")
                        with nc.allow_low_precision(reason="softmax denom bcast"):
                            nc.vector.reciprocal(rsb[:], pv[HD : HD + 1, :])
                        bc = ps_qkv.tile([64, 512], F32, tag="qkv", name="bc")
                        nc.tensor.matmul(
                            bc[:],
                            lhsT=ones[0:1, 0:64],
                            rhs=rsb[:],
                            start=True,
                            stop=True,
                        )
                        bcs = p_small.tile([64, 512], BF16, tag="bcs")
                        nc.vector.tensor_copy(bcs[:], bc[:])
                        nc.vector.tensor_mul(
                            YT[hp][
                                64 * h2 : 64 * h2 + 64,
                                i_base : i_base + 512,
                            ],
                            pv[0:HD, :],
                            bcs[:],
                        )

            prev = None
            prev_ps = None
            for t in triples:
                cur_ps = emit_s(t)
                if prev is not None:
                    emit_rest(prev, prev_ps)
                    php, pib, pjt = prev
                    if php == N_HP - 1 and pjt == 4 * (pib + 1) - 1 and use_pv:
                        emit_proj(pib)
                prev, prev_ps = t, cur_ps
            if prev is not None:
                emit_rest(prev, prev_ps)
                if use_pv:
                    emit_proj(prev[1])
            if not use_pv:
                # timing-ablation path: still produce all output blocks
                for tb in range(N_TB):
                    emit_proj(tb)

    nc.compile()
    return nc


_NC = {}


def _get_nc(with_bias: bool = True):
    key = bool(with_bias)
    if key not in _NC:
        _NC[key] = build_nc(with_bias=key)
    return _NC[key]


def make_in_maps(x, W_attn, b_attn, W_proj):
    bf = ml_dtypes.bfloat16
    xt_b = [np.ascontiguousarray(x[b].astype(bf).T) for b in range(B)]
    in_maps = []
    for core in range(8):
        b, g = core // 2, core % 2
        wq = W_attn[:, 512 * g : 512 * g + 512]
        wk = W_attn[:, C + 512 * g : C + 512 * g + 512]
        wv = W_attn[:, 2 * C + 512 * g : 2 * C + 512 * g + 512]
        wt = np.ascontiguousarray(np.concatenate([wq, wk, wv], axis=1)).astype(bf)
        bq = b_attn[512 * g : 512 * g + 512]
        bk = b_attn[C + 512 * g : C + 512 * g + 512]
        bv = b_attn[2 * C + 512 * g : 2 * C + 512 * g + 512]
        wb_ = np.concatenate([bq, bk, bv])[None, :].astype(bf)
        wp_ = np.ascontiguousarray(W_proj[512 * g : 512 * g + 512, :]).astype(bf)
        in_maps.append({"xt": xt_b[b], "wt": wt, "wb": wb_, "wp": wp_})
    return in_maps


def combine_outputs(results, b_proj):
    out = np.empty((B, T, C), dtype=np.float32)
    for b in range(B):
        acc = results[2 * b]["outT"] + results[2 * b + 1]["outT"]
        out[b] = acc.T + b_proj[None, :]
    return out


def kernel(x, W_attn, b_attn, W_proj, b_proj):
    x = np.asarray(x, dtype=np.float32)
    W_attn = np.asarray(W_attn, dtype=np.float32)
    b_attn = np.asarray(b_attn, dtype=np.float32)
    W_proj = np.asarray(W_proj, dtype=np.float32)
    b_proj = np.asarray(b_proj, dtype=np.float32)

    nc = _get_nc(with_bias=True)
    in_maps = make_in_maps(x, W_attn, b_attn, W_proj)
    res = run_bass_kernel_spmd(nc, in_maps, core_ids=list(range(8)))
    return combine_outputs(res.results, b_proj)
